# revision 49
# baseline (speedup 1.0000x reference)
"""Trainium2 Bass kernel for nn_GateActivation (e3nn gate: 512x0e + 256x1o + 128x2e).

Strategy (v2: mixed fp8-DoubleRow / fp16):
  - Data-parallel over rows: 65536 rows -> 8 cores x 8192 rows; weights replicated.
  - Host transposes each shard to feature-major [1920, R] (l>0 irreps
    de-interleaved plane-major), so every matmul is weights-stationary.
  - PE uses fp8e4m3 DoubleRow matmuls (0.5 cyc/row = 4x the fp16 MAC rate)
    wherever the 2e-2 tolerance allows, with hi+lo residual splits to kill
    the ~2% e4m3 quantization error where it would be seen directly:
      pre silu block  : 3-term split  Wh(x8+xl8) + Wl x8          (6 DR/chunk)
      pre gate block  : 2-term W-split (Wh+Wl) x8                 (4 DR/chunk)
      pre v1          : 3-term split                               (3 DR/chunk)
      pre v2          : fp16 (contraction 128; DR gains nothing)
      post scalar     : 3-term split on (sc_hi, sc_lo) fp8 acts   (6 DR/chunk)
      post v1         : 3-term split on (v1g_hi, v1g_lo)          (3 DR/chunk)
      post v2         : fp16
    Measured end-to-end error of this mix on the real inputs: ~1.5e-3
    (vs 5.5e-4 all-fp16, tolerance 2e-2).
  - All fp8 weights are stored at NATURAL scale (std ~1) to stay clear of
    e4m3 subnormals; the e3nn 1/sqrt(fan_in) norms and the resulting
    sqrt(fan_in) excesses are folded into the ACT sigmoid input scale and
    the PSUM-evac scales (free: ACT activation(Copy, scale), DVE
    tensor_scalar mul).
  - On-chip fp8 act splits (sc, v1g) run on Pool/gpsimd (SBUF-only ops:
    copy f16->fp8 + sub f16,fp8->fp8 - GPSIMD cannot touch PSUM); the
    residual lo is computed from the actual on-chip hi, so the HW rounding
    mode of the hi cast cancels out.
  - Software pipeline: post(g-1) is emitted after pre(g), giving the
    sigmoid -> gate-mul -> Pool-split chain a full group period before its
    post matmuls issue; PE runs at 97%+ occupancy in steady state.
  - Engine budget per 512-row group (PE ~12.4us critical):
      PE   96 DR + 10 fp16 matmuls; ACT 7 sigmoids + 12 scaled evacs;
      DVE  15 gate/silu muls + 3 scaled evacs; Pool 4 wide hi/lo splits;
      SP   all DMA (~11.8us/group, x prefetched 2 groups ahead, store(g-1)
      issued after post(g-1); the last two groups' stores split SP/ACT).
  - Warm-up matmuls keep the PE p-state ramping (0.65 -> 2.4 GHz over 3us
    of continuous execution) until group 0's first real matmul; group 0's
    x/weight DMAs are ordered by first use so that handoff is gapless.
  Measured: 219.9us on 8 cores (CoreSim cost model; fp16 baseline 276.6us),
  rel err 1.17e-2 on HW vs the f64 reference (tolerance 2e-2).
"""

import os
import sys
from contextlib import ExitStack

import numpy as np

sys.path.insert(0, "/opt/trn_rl_repo")

import ml_dtypes  # noqa: E402

import concourse.bass as bass  # noqa: E402
import concourse.tile as tile  # noqa: E402
from concourse import bacc, mybir  # noqa: E402
from concourse.bass_utils import run_bass_kernel_spmd  # noqa: E402

# Problem shape (hardcoded per harness contract)
N_ROWS = 65536
N_CORES = 8
R = N_ROWS // N_CORES  # rows per core
D_IN = 1920
M0, M1, M2 = 512, 256, 128
GRP = 512  # rows per on-chip group (matmul moving free dim)

F32 = mybir.dt.float32
F16 = mybir.dt.float16
F8 = mybir.dt.float8e4
DR = mybir.MatmulPerfMode.DoubleRow
SIGMOID = mybir.ActivationFunctionType.Sigmoid
COPY = mybir.ActivationFunctionType.Copy
NPF8 = ml_dtypes.float8_e4m3fn

RS0 = float(1.0 / np.sqrt(np.float64(M0)))  # sigmoid input scale
EV0 = float(1.0 / np.float64(M0))           # post-scalar evac scale
EV1 = float(1.0 / np.float64(M1))
EV2 = float(1.0 / np.float64(M2))

# x chunk layout (fp8 tensor): 0-3 x8_s | 4-9 x8_v1 | 10-13 xl8_s | 14-19 xl8_v1
NCH8 = 20
NCH16 = 5  # x16 tensor: v2 planes

# wall8 column layout (all [128, 2, 128] DR blocks = 256 cols each, natural scale)
#   gate Wh (3m x 2kp) | gate Wl (3m x 2kp) | silu Wh (4m x 2kp) | silu Wl
#   | v1pre Wh (2c) | v1pre Wl | w0post Wh (4m x 2kp) | w0post Wl
#   | w1post Wh (2c) | w1post Wl
W8_GATE_H = 0
W8_GATE_L = W8_GATE_H + 3 * 2 * 256   # 1536
W8_SILU_H = W8_GATE_L + 3 * 2 * 256   # 3072
W8_SILU_L = W8_SILU_H + 4 * 2 * 256   # 5120
W8_V1_H = W8_SILU_L + 4 * 2 * 256     # 7168
W8_V1_L = W8_V1_H + 2 * 256           # 7680
W8_P0_H = W8_V1_L + 2 * 256           # 8192
W8_P0_L = W8_P0_H + 4 * 2 * 256       # 10240
W8_P1_H = W8_P0_L + 4 * 2 * 256       # 12288
W8_P1_L = W8_P1_H + 2 * 256           # 12800
W8_COLS = W8_P1_L + 2 * 256           # 13312

# wall16 layout: v2pre [128,128] | w2post [128,128]  (natural scale)
W16_COLS = 256

last_results = None


def build_nc(rows=R, grp=GRP):
    """Build the per-core Bass program (SPMD; same program on all 8 cores)."""
    assert rows % grp == 0
    n_groups = rows // grp
    nc = bacc.Bacc("TRN2", target_bir_lowering=False, debug=False)

    xT8 = nc.dram_tensor("xT8", [n_groups, 128, NCH8, grp], F8,
                         kind="ExternalInput")
    xT16 = nc.dram_tensor("xT16", [n_groups, 128, NCH16, grp], F16,
                          kind="ExternalInput")
    wall8 = nc.dram_tensor("wall8", [128, W8_COLS], F8, kind="ExternalInput")
    wall16 = nc.dram_tensor("wall16", [128, W16_COLS], F16,
                            kind="ExternalInput")
    outT = nc.dram_tensor("outT", [n_groups, 128, 15 * grp], F16,
                          kind="ExternalOutput")

    with TileKernel(nc) as tk:
        tk.emit(xT8, xT16, wall8, wall16, outT, n_groups, grp)
    nc.compile()
    return nc


class TileKernel:
    def __init__(self, nc):
        self.nc = nc
        self.ctx = ExitStack()

    def __enter__(self):
        self.tc = self.ctx.enter_context(tile.TileContext(self.nc))
        return self

    def __exit__(self, *exc):
        return self.ctx.__exit__(*exc)

    def emit(self, xT8, xT16, wall8, wall16, outT, n_groups, grp):
        nc, tc, ctx = self.nc, self.tc, self.ctx

        wpool = ctx.enter_context(tc.tile_pool(name="w", bufs=1))
        xpool = ctx.enter_context(tc.tile_pool(name="x", bufs=3))
        ypool = ctx.enter_context(tc.tile_pool(name="y", bufs=2))
        apool = ctx.enter_context(tc.tile_pool(name="act", bufs=2))
        pre_ps = ctx.enter_context(
            tc.tile_pool(name="pre_ps", bufs=4, space=bass.MemorySpace.PSUM))
        post_ps = ctx.enter_context(
            tc.tile_pool(name="post_ps", bufs=4, space=bass.MemorySpace.PSUM))

        # --- weights resident for the whole kernel, ordered by first use ---
        w8 = wpool.tile([128, W8_COLS], F8, tag="w8")
        w16 = wpool.tile([128, W16_COLS], F16, tag="w16")
        # gate weights first: group 0's first matmuls start after ~2.5us
        nc.sync.dma_start(w8[:, :W8_GATE_L], wall8[:, :W8_GATE_L])
        nc.sync.dma_start(w8[:, W8_GATE_L:W8_SILU_H],
                          wall8[:, W8_GATE_L:W8_SILU_H])

        def load_x(g):
            x8 = xpool.tile([128, NCH8, grp], F8, tag="x8")
            nc.sync.dma_start(x8[:], xT8[g, :, :, :])
            x16 = xpool.tile([128, NCH16, grp], F16, tag="x16")
            nc.sync.dma_start(x16[:], xT16[g, :, :, :])
            return x8, x16

        def store_y(g, yts, split=False):
            # Split stores across the SP and ACT HWDGE queues for the tail
            # groups (output DMA can no longer hide behind later compute).
            yt_a, yt_b = yts
            eng2 = nc.scalar if split else nc.sync
            eng2.dma_start(outT[g, :, :4 * grp], yt_a[:, :4 * grp])
            eng2.dma_start(outT[g, :, 4 * grp:8 * grp], yt_a[:, 4 * grp:])
            nc.sync.dma_start(outT[g, :, 8 * grp:12 * grp], yt_b[:, :4 * grp])
            nc.sync.dma_start(outT[g, :, 12 * grp:], yt_b[:, 4 * grp:])

        # group 0's x in first-use order: scalar x8 (gate), scalar xl8
        # (silu), then the rest; remaining weights interleave behind.
        x8_0 = xpool.tile([128, NCH8, grp], F8, tag="x8")
        nc.sync.dma_start(x8_0[:, 0:4], xT8[0, :, 0:4, :])
        nc.sync.dma_start(w8[:, W8_SILU_H:W8_V1_H], wall8[:, W8_SILU_H:W8_V1_H])
        nc.sync.dma_start(x8_0[:, 10:14], xT8[0, :, 10:14, :])
        nc.sync.dma_start(w8[:, W8_V1_H:W8_P0_H], wall8[:, W8_V1_H:W8_P0_H])
        nc.sync.dma_start(x8_0[:, 4:10], xT8[0, :, 4:10, :])
        nc.sync.dma_start(x8_0[:, 14:20], xT8[0, :, 14:20, :])
        x16_0 = xpool.tile([128, NCH16, grp], F16, tag="x16")
        nc.sync.dma_start(x16_0[:], xT16[0, :, :, :])
        nc.sync.dma_start(w16[:], wall16[:])
        nc.sync.dma_start(w8[:, W8_P0_H:], wall8[:, W8_P0_H:])
        xtiles = {0: (x8_0, x16_0)}
        if n_groups > 1:
            xtiles[1] = load_x(1)
        prev_y = None

        # PE warm-up: dummy matmuls keep the PE p-state ramping (and the
        # engine continuously busy) until group 0's first real matmul.
        warm_x = wpool.tile([128, grp], F16, tag="warm")
        nc.vector.memset(warm_x[:], 0)
        warm_ps = pre_ps.tile([128, grp], F32, tag="pre")
        for _ in range(10):
            nc.tensor.matmul(warm_ps[:], warm_x[:, :128], warm_x[:],
                             start=True, stop=True)

        # weight slice helpers: DR block j at base -> [128, 2, 128]
        def w8blk(base, j):
            return w8[:, base + j * 256:base + (j + 1) * 256].rearrange(
                "p (i m) -> p i m", i=2)

        w2pre = w16[:, 0:128]
        w2post = w16[:, 128:256]

        def emit_pre(g):
            """Pre stage of group g: fp8/f16 matmuls, sigmoids, gate muls,
            Pool hi/lo splits. Returns the act tiles post(g) will need."""
            x8, x16 = xtiles.pop(g)

            def xp8(c):  # chunk pair [128, 2, grp] starting at chunk c
                return x8[:, c:c + 2, :]

            # gate chunks m=0..2: 2-term W-split, contraction 4x256
            gates = apool.tile([128, 3, grp], F32, tag="gates")
            for m in range(3):
                ps = pre_ps.tile([128, grp], F32, tag="pre")
                for t, base in enumerate((W8_GATE_H, W8_GATE_L)):
                    for kp in range(2):
                        nc.tensor.matmul(
                            ps[:], w8blk(base, m * 2 + kp), xp8(2 * kp),
                            start=(t == 0 and kp == 0),
                            stop=(t == 1 and kp == 1), perf_mode=DR)
                nc.scalar.activation(gates[:, m, :], ps[:], SIGMOID, scale=RS0)

            # silu chunks m=0..3: 3-term split, contraction 6x256
            sc_t = apool.tile([128, 4, grp], F16, tag="sc_t")
            for m in range(4):
                ps = pre_ps.tile([128, grp], F32, tag="pre")
                k = 0
                for base, xbase in ((W8_SILU_H, 0), (W8_SILU_H, 10),
                                    (W8_SILU_L, 0)):
                    for kp in range(2):
                        nc.tensor.matmul(
                            ps[:], w8blk(base, m * 2 + kp), xp8(xbase + 2 * kp),
                            start=(k == 0), stop=(k == 5), perf_mode=DR)
                        k += 1
                sg = apool.tile([128, grp], F32, tag="sg")
                nc.scalar.activation(sg[:], ps[:], SIGMOID, scale=RS0)
                # sc_t = ps * sigmoid(ps/sqrt(512)) = sqrt(512)*silu(s_pre)
                nc.vector.tensor_mul(sc_t[:, m, :], ps[:], sg[:])

            # sc hi/lo fp8 split (Pool, SBUF-only, wide ops)
            sc_hi = apool.tile([128, 4, grp], F8, tag="sc_hi")
            sc_lo = apool.tile([128, 4, grp], F8, tag="sc_lo")
            nc.gpsimd.tensor_copy(sc_hi[:], sc_t[:])
            nc.gpsimd.tensor_sub(sc_lo[:], sc_t[:], sc_hi[:])

            # pre v1: 3-term split, contraction 3x256 per (plane, c)
            v1g_t = apool.tile([128, 3, 2, grp], F16, tag="v1g_t")
            for i in range(3):
                for c in range(2):
                    ps = pre_ps.tile([128, grp], F32, tag="pre")
                    k = 0
                    for base, xc in ((W8_V1_H, 4 + 2 * i), (W8_V1_H, 14 + 2 * i),
                                     (W8_V1_L, 4 + 2 * i)):
                        nc.tensor.matmul(
                            ps[:], w8blk(base, c), xp8(xc),
                            start=(k == 0), stop=(k == 2), perf_mode=DR)
                        k += 1
                    nc.vector.tensor_mul(v1g_t[:, i, c, :], ps[:],
                                         gates[:, c, :])

            v1g_hi = apool.tile([128, 3, 2, grp], F8, tag="v1g_hi")
            v1g_lo = apool.tile([128, 3, 2, grp], F8, tag="v1g_lo")
            nc.gpsimd.tensor_copy(v1g_hi[:], v1g_t[:])
            nc.gpsimd.tensor_sub(v1g_lo[:], v1g_t[:], v1g_hi[:])

            # pre v2 (fp16) + gating
            v2g_t = apool.tile([128, 5, grp], F16, tag="v2g_t")
            for i in range(5):
                ps = pre_ps.tile([128, grp], F32, tag="pre")
                nc.tensor.matmul(ps[:], w2pre, x16[:, i, :],
                                 start=True, stop=True)
                nc.vector.tensor_mul(v2g_t[:, i, :], ps[:], gates[:, 2, :])

            return sc_hi, sc_lo, v1g_hi, v1g_lo, v2g_t

        def emit_post(acts, store_g=None):
            """Post stage: consumes act tiles, returns y tiles. When
            store_g is set (final group), stores are issued inline right
            after their source evacs: early chunks on the ACT queue, late
            chunks on SP, so the tail drain overlaps the remaining PE/ACT
            work instead of serializing after it."""
            sc_hi, sc_lo, v1g_hi, v1g_lo, v2g_t = acts
            yt_a = ypool.tile([128, 8 * grp], F16, tag="yt_a")
            yt_b = ypool.tile([128, 7 * grp], F16, tag="yt_b")

            def ytc(chunk):
                if chunk < 8:
                    return yt_a[:, chunk * grp:(chunk + 1) * grp]
                return yt_b[:, (chunk - 8) * grp:(chunk - 7) * grp]

            def evac(ps, chunk, scale, on_dve):
                if on_dve:
                    nc.vector.tensor_scalar_mul(ytc(chunk), ps[:], scale)
                else:
                    nc.scalar.activation(ytc(chunk), ps[:], COPY, scale=scale)

            # post-scalar: 3-term on (sc_hi, sc_lo), contraction 6x256
            for m in range(4):
                ps = post_ps.tile([128, grp], F32, tag="post")
                k = 0
                for base, act in ((W8_P0_H, sc_hi), (W8_P0_H, sc_lo),
                                  (W8_P0_L, sc_hi)):
                    for kp in range(2):
                        nc.tensor.matmul(
                            ps[:], w8blk(base, m * 2 + kp),
                            act[:, 2 * kp:2 * kp + 2, :],
                            start=(k == 0), stop=(k == 5), perf_mode=DR)
                        k += 1
                evac(ps, m, EV0, on_dve=(m == 3))
            if store_g is not None:
                nc.sync.dma_start(outT[store_g, :, :4 * grp],
                                  yt_a[:, :4 * grp])

            # post v1: 3-term, contraction 3x256 per (plane, c_out)
            for i in range(3):
                for c in range(2):
                    ps = post_ps.tile([128, grp], F32, tag="post")
                    k = 0
                    for base, act in ((W8_P1_H, v1g_hi), (W8_P1_H, v1g_lo),
                                      (W8_P1_L, v1g_hi)):
                        nc.tensor.matmul(
                            ps[:], w8blk(base, c), act[:, i, :, :],
                            start=(k == 0), stop=(k == 2), perf_mode=DR)
                        k += 1
                    evac(ps, 4 + 2 * i + c, EV1, on_dve=(i == 2 and c == 1))
            if store_g is not None:
                nc.sync.dma_start(outT[store_g, :, 4 * grp:8 * grp],
                                  yt_a[:, 4 * grp:])

            # post v2 (fp16)
            for i in range(5):
                ps = post_ps.tile([128, grp], F32, tag="post")
                nc.tensor.matmul(ps[:], w2post, v2g_t[:, i, :],
                                 start=True, stop=True)
                evac(ps, 10 + i, EV2, on_dve=(i == 4))
            if store_g is not None:
                nc.sync.dma_start(outT[store_g, :, 8 * grp:12 * grp],
                                  yt_b[:, :4 * grp])
                nc.sync.dma_start(outT[store_g, :, 12 * grp:],
                                  yt_b[:, 4 * grp:])

            return yt_a, yt_b

        # Software pipeline: post(g-1) runs after pre(g), so the
        # sigmoid->mul->Pool-split chain of group g-1 has a full group
        # period to complete before its post matmuls are issued.
        prev_acts = None
        for g in range(n_groups):
            # SP order per iteration: [in g+2][out g-1] — the store is
            # issued right after post(g-1) so its waits resolve during this
            # iteration and only one group of output drains at the end.
            if g + 2 < n_groups:
                xtiles[g + 2] = load_x(g + 2)
            acts = emit_pre(g)
            if prev_acts is not None:
                y = emit_post(prev_acts)
                if g == n_groups - 1:
                    penult_y = y  # deferred: stored after post(n-1) below
                else:
                    store_y(g - 1, y)
            prev_acts = acts

        emit_post(prev_acts, store_g=n_groups - 1)
        # penultimate group's stores go last, split across the now-idle ACT
        # queue and SP, overlapping the final group's SP stores.
        yt_a, yt_b = penult_y
        g = n_groups - 2
        nc.scalar.dma_start(outT[g, :, :4 * grp], yt_a[:, :4 * grp])
        nc.scalar.dma_start(outT[g, :, 4 * grp:8 * grp], yt_a[:, 4 * grp:])
        nc.sync.dma_start(outT[g, :, 8 * grp:12 * grp], yt_b[:, :4 * grp])
        nc.sync.dma_start(outT[g, :, 12 * grp:], yt_b[:, 4 * grp:])


# ---------------------------------------------------------------------------
# Host-side layout transforms
# ---------------------------------------------------------------------------

def _fm_scalar(xs):
    """[r, 1920] -> scalar block feature-major [512, r]."""
    return np.ascontiguousarray(xs[:, :M0].T)


def _fm_v1(xs):
    r = xs.shape[0]
    return np.ascontiguousarray(
        xs[:, M0:M0 + 3 * M1].reshape(r, M1, 3).transpose(2, 1, 0).reshape(
            3 * M1, r))


def _fm_v2(xs):
    r = xs.shape[0]
    return np.ascontiguousarray(
        xs[:, M0 + 3 * M1:].reshape(r, M2, 5).transpose(2, 1, 0).reshape(
            5 * M2, r))


def _to_chunks(fm, grp):
    """[F, r] -> [F//128 chunks, G groups] -> [G, 128, C, grp] block view."""
    F, r = fm.shape
    c = F // 128
    g = r // grp
    return fm.reshape(c, 128, g, grp).transpose(2, 1, 0, 3)


def make_core_inputs(shard, wmaps, grp=GRP):
    """Full per-core input map for a [rows, 1920] float32 shard."""
    xs = np.asarray(shard, np.float32)
    r = xs.shape[0]
    g = r // grp

    s_fm = _fm_scalar(xs)        # [512, r]
    v1_fm = _fm_v1(xs)           # [768, r]
    v2_fm = _fm_v2(xs)           # [640, r]

    s8 = s_fm.astype(NPF8)
    sl8 = (s_fm - s8.astype(np.float32)).astype(NPF8)
    v18 = v1_fm.astype(NPF8)
    v1l8 = (v1_fm - v18.astype(np.float32)).astype(NPF8)

    x8 = np.empty((g, 128, NCH8, grp), NPF8)
    x8[:, :, 0:4] = _to_chunks(s8, grp)
    x8[:, :, 4:10] = _to_chunks(v18, grp)
    x8[:, :, 10:14] = _to_chunks(sl8, grp)
    x8[:, :, 14:20] = _to_chunks(v1l8, grp)

    x16 = np.ascontiguousarray(
        _to_chunks(v2_fm.astype(np.float16), grp))

    m = {"xT8": np.ascontiguousarray(x8),
         "xT16": np.ascontiguousarray(x16)}
    m.update(wmaps)
    return m


def _dr_blocks(W, n_m):
    """[K, M] natural-scale -> fp8 hi/lo DR blocks [128, n_blk*256].

    Block order: m-chunk major, k-pair minor: (m, kp) -> [128, 2, 128] with
    (p, i, mo) = W[kp*256 + i*128 + p, m*128 + mo].
    """
    K, M = W.shape
    n_kp = K // 256
    assert M == n_m * 128

    def pack(Wq):
        # [K, M] -> [kp, i, p, m_chunk, mo] -> [p, (m_chunk, kp, i, mo)]
        a = Wq.reshape(n_kp, 2, 128, n_m, 128)
        return a.transpose(2, 3, 0, 1, 4).reshape(128, n_m * n_kp * 256)

    hi = W.astype(NPF8)
    lo = (W - hi.astype(np.float32)).astype(NPF8)
    return pack(hi), pack(lo)


def prep_weights(W0_pre, W1_pre, W2_pre, W0_post, W1_post, W2_post):
    W0_pre = np.asarray(W0_pre, np.float32)
    gate_h, gate_l = _dr_blocks(W0_pre[:, M0:], 3)
    silu_h, silu_l = _dr_blocks(W0_pre[:, :M0], 4)
    v1_h, v1_l = _dr_blocks(np.asarray(W1_pre, np.float32), 2)
    p0_h, p0_l = _dr_blocks(np.asarray(W0_post, np.float32), 4)
    p1_h, p1_l = _dr_blocks(np.asarray(W1_post, np.float32), 2)

    wall8 = np.concatenate(
        [gate_h, gate_l, silu_h, silu_l, v1_h, v1_l, p0_h, p0_l, p1_h, p1_l],
        axis=1)
    assert wall8.shape == (128, W8_COLS), wall8.shape

    wall16 = np.concatenate(
        [np.asarray(W2_pre, np.float32).astype(np.float16),
         np.asarray(W2_post, np.float32).astype(np.float16)], axis=1)
    assert wall16.shape == (128, W16_COLS), wall16.shape
    return {"wall8": wall8, "wall16": wall16}


def decode_core_output(outT, rows, grp=GRP):
    """[G, 128, 15*grp] f16 device output -> [rows, 1920] float32."""
    g = rows // grp
    yT = np.asarray(outT).astype(np.float32).reshape(g, 128, 15, grp)
    yT = yT.transpose(2, 1, 0, 3).reshape(15 * 128, rows)
    out = np.empty((rows, D_IN), np.float32)
    out[:, :M0] = yT[:M0].T
    out[:, M0:M0 + 3 * M1] = (
        yT[M0:M0 + 3 * M1].reshape(3, M1, rows).transpose(2, 1, 0).reshape(
            rows, 3 * M1))
    out[:, M0 + 3 * M1:] = (
        yT[M0 + 3 * M1:].reshape(5, M2, rows).transpose(2, 1, 0).reshape(
            rows, 5 * M2))
    return out


_nc_cache = {}


def _get_nc(rows=R):
    if rows not in _nc_cache:
        _nc_cache[rows] = build_nc(rows=rows)
    return _nc_cache[rows]


def kernel(x, W0_pre, W1_pre, W2_pre, W0_post, W1_post, W2_post):
    global last_results
    x = np.asarray(x, dtype=np.float32)
    assert x.shape == (N_ROWS, D_IN), x.shape

    wmaps = prep_weights(W0_pre, W1_pre, W2_pre, W0_post, W1_post, W2_post)

    nc = _get_nc()
    in_maps = [make_core_inputs(x[c * R:(c + 1) * R], wmaps)
               for c in range(N_CORES)]

    trace = os.environ.get("BASS_GATE_TRACE", "0") == "1"
    last_results = run_bass_kernel_spmd(
        nc, in_maps, list(range(N_CORES)), trace=trace)

    out = np.empty((N_ROWS, D_IN), np.float32)
    for c in range(N_CORES):
        out[c * R:(c + 1) * R] = decode_core_output(
            last_results.results[c]["outT"], R)
    return out
